# revision 11
# baseline (speedup 1.0000x reference)
"""Trainium2 Bass kernel for nn_Net_36524401885621 (dense_cnn).

Model (per batch row b):
    h   = emb[x[b]]                      # [S, 300] gather
    f   = h.flatten()                    # [S*300]
    y1  = maxpool300(relu(conv5(f)))     # [32, 2047]
    y2  = maxpool1500(relu(conv5(y1)))   # [128, 1]
    out = softmax(y2 @ wl.T + bl)        # [5]

Key structure exploited:
  - Only conv2 output positions 0..1499 survive the k=1500 maxpool, so only
    y1[:, 0:1504] and hence tokens x[b, 0:1505] are ever needed.
  - maxpool window (300) == embedding dim: window s covers emb rows s and
    the first 4 elements of row s+1.
  - relu commutes with max, so relu is applied after the max reduction.
  - conv1 (Cin=1, K=5) is computed on the tensor engine as matmuls with a
    banded Toeplitz "moving" matrix against transposed embedding blocks as
    the stationary operand; outputs land [token-partition, (ch,d)-free] so
    the 300-way maxpool is a free-dim max reduction.

Sharding: pure data parallelism — core b computes batch row b; the (bf16)
embedding table and all weights are replicated.
"""

import os
import sys

import numpy as np

sys.path.insert(0, "/opt/trn_rl_repo")

import ml_dtypes  # noqa: E402

import concourse.bass as bass  # noqa: E402
import concourse.tile as tile  # noqa: E402
from concourse import bacc, mybir  # noqa: E402
from concourse.bass_utils import run_bass_kernel_spmd  # noqa: E402
from concourse.masks import make_identity  # noqa: E402

BF16 = mybir.dt.bfloat16
F32 = mybir.dt.float32
I32 = mybir.dt.int32

B, SEQ, VOCAB, D = 8, 2048, 50000, 300
C1, C2, K = 32, 128, 5
NTILES = 12          # token tiles of 128 -> covers tokens 0..1535 (need 0..1504)
TOKPAD = NTILES * 128
NEG = -1.0e30        # max-tree padding value

TRACE = False        # test.py sets True to capture an NTFF profile
DEBUG = False        # test.py sets True to dump intermediates
LAST_RESULT = None   # BassKernelResults of the last run (for test.py)

_cached = {}


def _build_program(debug=False):
    nc = bacc.Bacc("TRN2", target_bir_lowering=False, debug=False, num_devices=8)

    emb16 = nc.dram_tensor("emb16", [VOCAB, D], BF16, kind="ExternalInput").ap()
    emb4 = nc.dram_tensor("emb4", [VOCAB, 4], BF16, kind="ExternalInput").ap()
    xi_d = nc.dram_tensor("xi", [128, NTILES], I32, kind="ExternalInput").ap()
    xn_d = nc.dram_tensor("xn", [128, NTILES], I32, kind="ExternalInput").ap()
    r124_d = nc.dram_tensor("r124", [128, C1 * 124], BF16, kind="ExternalInput").ap()
    r52_d = nc.dram_tensor("r52", [56, C1 * 52], BF16, kind="ExternalInput").ap()
    w2t_d = nc.dram_tensor("w2t", [C1, K * C2], BF16, kind="ExternalInput").ap()
    wl_d = nc.dram_tensor("wl", [C2, 5], BF16, kind="ExternalInput").ap()
    b1bc_d = nc.dram_tensor("b1bc", [128, C1], F32, kind="ExternalInput").ap()
    b2c_d = nc.dram_tensor("b2c", [128, 1], F32, kind="ExternalInput").ap()
    blc_d = nc.dram_tensor("blc", [1, 5], F32, kind="ExternalInput").ap()
    out_d = nc.dram_tensor("out", [1, 5], F32, kind="ExternalOutput").ap()
    if debug:
        dbg_e = nc.dram_tensor("dbg_e", [128, 304], BF16, kind="ExternalOutput").ap()
        dbg_v = nc.dram_tensor("dbg_v", [128, 384], BF16, kind="ExternalOutput").ap()
        dbg_x = nc.dram_tensor("dbg_x", [128, 8192], BF16, kind="ExternalOutput").ap()
        dbg_w2 = nc.dram_tensor("dbg_w2", [128, 32], F32, kind="ExternalOutput").ap()
        dbg_o1 = nc.dram_tensor(
            "dbg_o1", [128, NTILES * C1], BF16, kind="ExternalOutput"
        ).ap()
        dbg_o1t = nc.dram_tensor(
            "dbg_o1t", [C1, TOKPAD], BF16, kind="ExternalOutput"
        ).ap()
        dbg_hp = nc.dram_tensor("dbg_hp", [128, 4], F32, kind="ExternalOutput").ap()
        dbg_lg = nc.dram_tensor("dbg_lg", [1, 8], F32, kind="ExternalOutput").ap()

    with tile.TileContext(nc) as tc:
        import contextlib

        ctx = contextlib.ExitStack()
        with ctx:
            consts = ctx.enter_context(tc.tile_pool(name="consts", bufs=1))
            e_pool = ctx.enter_context(tc.tile_pool(name="epool", bufs=3))
            vsb_pool = ctx.enter_context(tc.tile_pool(name="vsb", bufs=2))
            x_pool = ctx.enter_context(tc.tile_pool(name="xbuf", bufs=1))
            tree_pool = ctx.enter_context(tc.tile_pool(name="tree", bufs=2))
            small_pool = ctx.enter_context(tc.tile_pool(name="small", bufs=2))
            vp_pool = ctx.enter_context(
                tc.tile_pool(name="vpsum", bufs=1, space="PSUM")
            )
            conv_pool = ctx.enter_context(
                tc.tile_pool(name="convpsum", bufs=2, space="PSUM")
            )
            w2p_pool = ctx.enter_context(
                tc.tile_pool(name="w2psum", bufs=2, space="PSUM")
            )
            misc_psum = ctx.enter_context(
                tc.tile_pool(name="miscpsum", bufs=1, space="PSUM")
            )

            # ---- load constants ----
            r124 = consts.tile([128, C1 * 124], BF16)
            nc.sync.dma_start(r124[:], r124_d)
            r52 = consts.tile([56, C1 * 52], BF16)
            nc.sync.dma_start(r52[:], r52_d)
            xi = consts.tile([128, NTILES], I32)
            nc.sync.dma_start(xi[:], xi_d)
            xn = consts.tile([128, NTILES], I32)
            nc.sync.dma_start(xn[:], xn_d)
            w2t = consts.tile([C1, K * C2], BF16)
            nc.sync.dma_start(w2t[:], w2t_d)
            wl = consts.tile([C2, 5], BF16)
            nc.sync.dma_start(wl[:], wl_d)
            b1bc = consts.tile([128, C1], F32)
            nc.sync.dma_start(b1bc[:], b1bc_d)
            b2c = consts.tile([128, 1], F32)
            nc.sync.dma_start(b2c[:], b2c_d)
            blc = consts.tile([1, 5], F32)
            nc.sync.dma_start(blc[:], blc_d)

            ident = consts.tile([128, 128], BF16)
            make_identity(nc, ident[:])

            # persistent ping-pong max-tree buffers, padded with NEG
            xbuf0 = x_pool.tile([128, C1 * 256], BF16)
            xbuf1 = x_pool.tile([128, C1 * 256], BF16)
            nc.gpsimd.memset(xbuf0[:], NEG)
            nc.gpsimd.memset(xbuf1[:], NEG)

            out1_all = consts.tile([128, NTILES * C1], BF16)
            out1_3 = out1_all[:].rearrange("p (t c) -> p t c", t=NTILES)

            # ---- conv1 + maxpool over 12 token tiles ----
            for i in range(NTILES):
                E = e_pool.tile([128, 304], BF16, tag="E")
                nc.gpsimd.indirect_dma_start(
                    out=E[:, 0:300],
                    out_offset=None,
                    in_=emb16,
                    in_offset=bass.IndirectOffsetOnAxis(ap=xi[:, i : i + 1], axis=0),
                )
                nc.gpsimd.indirect_dma_start(
                    out=E[:, 300:304],
                    out_offset=None,
                    in_=emb4,
                    in_offset=bass.IndirectOffsetOnAxis(ap=xn[:, i : i + 1], axis=0),
                )

                # transpose the three d-windows: V[d, tok]
                Vp = vp_pool.tile([128, 384], BF16, tag="vp", space="PSUM")
                nc.tensor.transpose(Vp[:, 0:128], E[:, 0:128], ident[:])
                nc.tensor.transpose(Vp[:, 128:256], E[:, 124:252], ident[:])
                nc.tensor.transpose(Vp[0:56, 256:384], E[:, 248:304], ident[:])
                V = vsb_pool.tile([128, 384], BF16, tag="V")
                nc.vector.tensor_copy(V[:, 0:256], Vp[:, 0:256])
                nc.vector.tensor_copy(V[0:56, 256:384], Vp[0:56, 256:384])

                xb = xbuf0 if i % 2 == 0 else xbuf1
                x3 = xb[:].rearrange("p (c d) -> p c d", c=C1)  # [128, 32, 256]

                w2dir = small_pool.tile([128, C1], F32, tag="w2dir")

                for u in range(4):  # 8-channel units
                    c0 = 8 * u
                    # window 0 (d 0..123) and window 1 (d 124..247)
                    P01 = conv_pool.tile([128, 1024], F32, tag="cp", space="PSUM")
                    nc.tensor.matmul(
                        P01[:, 0:496], V[:, 0:128],
                        r124[:, c0 * 124 : (c0 + 4) * 124], start=True, stop=True,
                    )
                    nc.tensor.matmul(
                        P01[:, 512:1008], V[:, 0:128],
                        r124[:, (c0 + 4) * 124 : (c0 + 8) * 124], start=True, stop=True,
                    )
                    P11 = conv_pool.tile([128, 1024], F32, tag="cp", space="PSUM")
                    nc.tensor.matmul(
                        P11[:, 0:496], V[:, 128:256],
                        r124[:, c0 * 124 : (c0 + 4) * 124], start=True, stop=True,
                    )
                    nc.tensor.matmul(
                        P11[:, 512:1008], V[:, 128:256],
                        r124[:, (c0 + 4) * 124 : (c0 + 8) * 124], start=True, stop=True,
                    )
                    # window 2 (d 248..299): 8 channels x 52
                    P2 = w2p_pool.tile([128, 512], F32, tag="w2p", space="PSUM")
                    nc.tensor.matmul(
                        P2[:, 0:416], V[0:56, 256:384],
                        r52[:, c0 * 52 : (c0 + 8) * 52], start=True, stop=True,
                    )

                    # drain: ACT copy-casts windows 0/1 into the tree buffer
                    # psum layout: [bank b=2][c=4 channels x d=124 compact][pad]
                    p01v = (
                        P01[:]
                        .rearrange("p (b x) -> p b x", b=2)[:, :, 0:496]
                        .rearrange("p b (c d) -> p b c d", c=4)
                    )
                    p11v = (
                        P11[:]
                        .rearrange("p (b x) -> p b x", b=2)[:, :, 0:496]
                        .rearrange("p b (c d) -> p b c d", c=4)
                    )
                    d0 = x3[:, c0 : c0 + 8, 0:124].rearrange(
                        "p (b c) d -> p b c d", b=2
                    )
                    d1 = x3[:, c0 : c0 + 8, 128:252].rearrange(
                        "p (b c) d -> p b c d", b=2
                    )
                    nc.scalar.copy(d0, p01v)
                    nc.scalar.copy(d1, p11v)
                    # DVE direct-reduces window 2 (8 channels x 52, compact)
                    p2v = P2[:, 0:416].rearrange("p (c d) -> p c d", c=8)
                    nc.vector.reduce_max(
                        w2dir[:, c0 : c0 + 8], p2v, axis=mybir.AxisListType.X,
                    )

                # max tree over d (bf16, SBUF, all 2x-mode)
                l1 = tree_pool.tile([128, C1 * 128], BF16, tag="l1")
                l1v = l1[:].rearrange("p (c d) -> p c d", c=C1)
                nc.vector.tensor_tensor(
                    l1v, x3[:, :, 0:128], x3[:, :, 128:256], op=mybir.AluOpType.max
                )
                l2 = tree_pool.tile([128, C1 * 64], BF16, tag="l2")
                l2v = l2[:].rearrange("p (c d) -> p c d", c=C1)
                nc.vector.tensor_tensor(
                    l2v, l1v[:, :, 0:64], l1v[:, :, 64:128], op=mybir.AluOpType.max
                )
                l3 = tree_pool.tile([128, C1 * 32], BF16, tag="l3")
                l3v = l3[:].rearrange("p (c d) -> p c d", c=C1)
                nc.vector.tensor_tensor(
                    l3v, l2v[:, :, 0:32], l2v[:, :, 32:64], op=mybir.AluOpType.max
                )
                l4 = tree_pool.tile([128, C1 * 16], BF16, tag="l4")
                l4v = l4[:].rearrange("p (c d) -> p c d", c=C1)
                nc.vector.tensor_tensor(
                    l4v, l3v[:, :, 0:16], l3v[:, :, 16:32], op=mybir.AluOpType.max
                )
                if debug and i == 0:
                    nc.sync.dma_start(dbg_e, E[:])
                    nc.sync.dma_start(dbg_v, V[:])
                    nc.sync.dma_start(dbg_x, xb[:])
                    nc.sync.dma_start(dbg_w2, w2dir[:])

                red = small_pool.tile([128, C1], BF16, tag="red")
                nc.vector.reduce_max(red[:], l4v, axis=mybir.AxisListType.X)

                cmb = small_pool.tile([128, C1], F32, tag="cmb")
                nc.vector.tensor_tensor(
                    cmb[:], red[:], w2dir[:], op=mybir.AluOpType.max
                )
                cmb2 = small_pool.tile([128, C1], F32, tag="cmb2")
                nc.vector.tensor_tensor(
                    cmb2[:], cmb[:], b1bc[:], op=mybir.AluOpType.add
                )
                nc.vector.tensor_scalar_max(out1_3[:, i, :], cmb2[:], 0.0)

            # ---- transpose out1 -> [32, 1536] ----
            out1t = consts.tile([C1, TOKPAD], BF16)
            for i in range(NTILES):
                Tp = misc_psum.tile([C1, 128], BF16, tag="mp", space="PSUM")
                nc.tensor.transpose(Tp[:], out1_3[:, i, :], ident[:])
                nc.vector.tensor_copy(out1t[:, i * 128 : (i + 1) * 128], Tp[:])

            # ---- conv2 + maxpool(1500) ----
            hpart = consts.tile([128, 4], F32)
            for j in range(3):
                Cp = misc_psum.tile([C2, 512], F32, tag="mp", space="PSUM")
                for k in range(K):
                    nc.tensor.matmul(
                        Cp[:, 0:500],
                        w2t[:, k * C2 : (k + 1) * C2],
                        out1t[:, j * 500 + k : j * 500 + k + 500],
                        start=(k == 0),
                        stop=(k == K - 1),
                    )
                nc.vector.reduce_max(
                    hpart[:, j : j + 1], Cp[:, 0:500], axis=mybir.AxisListType.X
                )

            if debug:
                nc.sync.dma_start(dbg_o1, out1_all[:])
                nc.sync.dma_start(dbg_o1t, out1t[:])
                nc.sync.dma_start(dbg_hp, hpart[:])

            hmax = consts.tile([128, 1], F32)
            nc.vector.reduce_max(hmax[:], hpart[:, 0:3], axis=mybir.AxisListType.X)
            hb = consts.tile([128, 1], F32)
            nc.vector.tensor_scalar_add(hb[:], hmax[:], b2c[:, 0:1])
            h16 = consts.tile([128, 1], BF16)
            nc.vector.tensor_scalar_max(h16[:], hb[:], 0.0)

            # ---- linear + softmax ----
            Fp = misc_psum.tile([1, 512], F32, tag="mp", space="PSUM")
            nc.tensor.matmul(Fp[0:1, 0:5], h16[:, 0:1], wl[:, 0:5], start=True, stop=True)
            lg = consts.tile([1, 8], F32)
            nc.vector.tensor_tensor(
                lg[:, 0:5], Fp[0:1, 0:5], blc[0:1, 0:5], op=mybir.AluOpType.add
            )
            mx = consts.tile([1, 1], F32)
            nc.vector.reduce_max(mx[:], lg[:, 0:5], axis=mybir.AxisListType.X)
            sh = consts.tile([1, 8], F32)
            nc.vector.tensor_scalar(
                sh[:, 0:5], lg[:, 0:5], mx[0:1, 0:1], None,
                op0=mybir.AluOpType.subtract,
            )
            ex = consts.tile([1, 8], F32)
            nc.scalar.activation(
                ex[:, 0:5], sh[:, 0:5], mybir.ActivationFunctionType.Exp
            )
            sm = consts.tile([1, 1], F32)
            nc.vector.reduce_sum(sm[:], ex[:, 0:5], axis=mybir.AxisListType.X)
            rc = consts.tile([1, 1], F32)
            nc.vector.reciprocal(rc[:], sm[:])
            res = consts.tile([1, 8], F32)
            nc.vector.tensor_scalar_mul(res[:, 0:5], ex[:, 0:5], rc[0:1, 0:1])
            nc.sync.dma_start(out_d, res[:, 0:5])
            if debug:
                nc.sync.dma_start(dbg_lg, lg[:])

    nc.compile()
    return nc


def _prep_shared(emb, w1, b1, w2, b2, wl, bl):
    bf = ml_dtypes.bfloat16
    emb = np.asarray(emb, np.float32)
    emb16 = emb.astype(bf)
    emb4 = np.ascontiguousarray(emb16[:, 0:4])

    w1 = np.asarray(w1, np.float32).reshape(C1, K)
    r124 = np.zeros((128, C1 * 124), np.float32)
    i_idx = np.arange(128)[:, None]
    for c in range(C1):
        for d in range(124):
            for k in range(K):
                r124[d + k, c * 124 + d] = w1[c, k]
    r52 = np.zeros((56, C1 * 52), np.float32)
    for c in range(C1):
        for d in range(52):
            for k in range(K):
                r52[d + k, c * 52 + d] = w1[c, k]

    w2 = np.asarray(w2, np.float32)  # [C2, C1, K]
    w2t = np.zeros((C1, K * C2), np.float32)
    for k in range(K):
        w2t[:, k * C2 : (k + 1) * C2] = w2[:, :, k].T
    wlT = np.asarray(wl, np.float32).T  # [C2, 5]
    b1bc = np.broadcast_to(np.asarray(b1, np.float32)[None, :], (128, C1)).copy()
    b2c = np.broadcast_to(
        np.asarray(b2, np.float32)[:, None], (C2, 1)
    ).copy()  # [128,1] per-partition
    blc = np.asarray(bl, np.float32).reshape(1, 5).copy()

    return {
        "emb16": np.ascontiguousarray(emb16),
        "emb4": emb4,
        "r124": r124.astype(bf),
        "r52": r52.astype(bf),
        "w2t": w2t.astype(bf),
        "wl": wlT.astype(bf),
        "b1bc": b1bc,
        "b2c": b2c,
        "blc": blc,
    }


def kernel(**inputs) -> np.ndarray:
    global LAST_RESULT
    x = np.asarray(inputs["x"]).astype(np.int64)
    shared = _prep_shared(
        inputs["emb"], inputs["w1"], inputs["b1"], inputs["w2"],
        inputs["b2"], inputs["wl"], inputs["bl"],
    )

    key = ("dbg" if DEBUG else "nc")
    if key not in _cached:
        _cached[key] = _build_program(debug=DEBUG)
    nc = _cached[key]

    in_maps = []
    for b in range(B):
        xi = x[b, 0:TOKPAD].reshape(NTILES, 128).T.astype(np.int32)
        xn = x[b, 1 : TOKPAD + 1].reshape(NTILES, 128).T.astype(np.int32)
        m = dict(shared)
        m["xi"] = np.ascontiguousarray(xi)
        m["xn"] = np.ascontiguousarray(xn)
        in_maps.append(m)

    r = run_bass_kernel_spmd(nc, in_maps, list(range(B)), trace=TRACE)
    LAST_RESULT = r
    out = np.stack([np.asarray(r.results[b]["out"], np.float32)[0] for b in range(B)])
    return out


if __name__ == "__main__":
    rng = np.random.default_rng(0)
    ins = {
        "x": rng.integers(0, VOCAB, (B, SEQ)),
        "emb": rng.standard_normal((VOCAB, D), np.float32) * 0.1,
        "w1": rng.standard_normal((C1, 1, K), np.float32) * 0.1,
        "b1": np.zeros(C1, np.float32),
        "w2": rng.standard_normal((C2, C1, K), np.float32) * 0.05,
        "b2": np.zeros(C2, np.float32),
        "wl": rng.standard_normal((5, C2), np.float32) * 0.1,
        "bl": np.zeros(5, np.float32),
    }
    print(kernel(**ins))


# revision 17
# speedup vs baseline: 1.0352x; 1.0352x over previous
"""Trainium2 Bass kernel for nn_Net_36524401885621 (dense_cnn).

Model (per batch row b):
    h   = emb[x[b]]                      # [S, 300] gather
    f   = h.flatten()                    # [S*300]
    y1  = maxpool300(relu(conv5(f)))     # [32, 2047]
    y2  = maxpool1500(relu(conv5(y1)))   # [128, 1]
    out = softmax(y2 @ wl.T + bl)        # [5]

Key structure exploited:
  - Only conv2 output positions 0..1499 survive the k=1500 maxpool, so only
    y1[:, 0:1504] and hence tokens x[b, 0:1505] are ever needed.
  - maxpool window (300) == embedding dim: window s covers emb rows s and
    the first 4 elements of row s+1.
  - relu commutes with max, so relu is applied after the max reduction.
  - conv1 (Cin=1, K=5) is computed on the tensor engine as matmuls with a
    banded Toeplitz "moving" matrix against transposed embedding blocks as
    the stationary operand; outputs land [token-partition, (ch,d)-free] so
    the 300-way maxpool is a free-dim max reduction.

Sharding: pure data parallelism — core b computes batch row b; the (bf16)
embedding table and all weights are replicated.
"""

import os
import sys

import numpy as np

sys.path.insert(0, "/opt/trn_rl_repo")

import ml_dtypes  # noqa: E402

import concourse.bass as bass  # noqa: E402
import concourse.tile as tile  # noqa: E402
from concourse import bacc, mybir  # noqa: E402
from concourse.bass_utils import run_bass_kernel_spmd  # noqa: E402
from concourse.masks import make_identity  # noqa: E402

BF16 = mybir.dt.bfloat16
F32 = mybir.dt.float32
I32 = mybir.dt.int32

B, SEQ, VOCAB, D = 8, 2048, 50000, 300
C1, C2, K = 32, 128, 5
NTILES = 12          # token tiles of 128 -> covers tokens 0..1535 (need 0..1504)
TOKPAD = NTILES * 128
NEG = -1.0e30        # max-tree padding value

TRACE = False        # test.py sets True to capture an NTFF profile
DEBUG = False        # test.py sets True to dump intermediates
LAST_RESULT = None   # BassKernelResults of the last run (for test.py)

_cached = {}


def _build_program(debug=False):
    nc = bacc.Bacc("TRN2", target_bir_lowering=False, debug=False, num_devices=8)

    emb16 = nc.dram_tensor("emb16", [VOCAB, D], BF16, kind="ExternalInput").ap()
    emb4 = nc.dram_tensor("emb4", [VOCAB, 4], BF16, kind="ExternalInput").ap()
    xi_d = nc.dram_tensor("xi", [128, NTILES], I32, kind="ExternalInput").ap()
    xn_d = nc.dram_tensor("xn", [128, NTILES], I32, kind="ExternalInput").ap()
    r124_d = nc.dram_tensor("r124", [128, C1 * 124], BF16, kind="ExternalInput").ap()
    r52_d = nc.dram_tensor("r52", [56, C1 * 52], BF16, kind="ExternalInput").ap()
    w2t_d = nc.dram_tensor("w2t", [C1, K * C2], BF16, kind="ExternalInput").ap()
    wl_d = nc.dram_tensor("wl", [C2, 5], BF16, kind="ExternalInput").ap()
    b1bc_d = nc.dram_tensor("b1bc", [128, C1], F32, kind="ExternalInput").ap()
    b2c_d = nc.dram_tensor("b2c", [128, 1], F32, kind="ExternalInput").ap()
    blc_d = nc.dram_tensor("blc", [1, 5], F32, kind="ExternalInput").ap()
    out_d = nc.dram_tensor("out", [1, 5], F32, kind="ExternalOutput").ap()
    if debug:
        dbg_e = nc.dram_tensor("dbg_e", [128, 304], BF16, kind="ExternalOutput").ap()
        dbg_v = nc.dram_tensor("dbg_v", [128, 384], BF16, kind="ExternalOutput").ap()
        dbg_x = nc.dram_tensor("dbg_x", [128, 8192], BF16, kind="ExternalOutput").ap()
        dbg_w2 = nc.dram_tensor("dbg_w2", [128, 32], F32, kind="ExternalOutput").ap()
        dbg_o1 = nc.dram_tensor(
            "dbg_o1", [128, NTILES * C1], BF16, kind="ExternalOutput"
        ).ap()
        dbg_o1t = nc.dram_tensor(
            "dbg_o1t", [C1, TOKPAD], BF16, kind="ExternalOutput"
        ).ap()
        dbg_hp = nc.dram_tensor("dbg_hp", [128, 4], F32, kind="ExternalOutput").ap()
        dbg_lg = nc.dram_tensor("dbg_lg", [1, 8], F32, kind="ExternalOutput").ap()

    with tile.TileContext(nc) as tc:
        import contextlib

        ctx = contextlib.ExitStack()
        with ctx:
            consts = ctx.enter_context(tc.tile_pool(name="consts", bufs=1))
            e_pool = ctx.enter_context(tc.tile_pool(name="epool", bufs=3))
            vsb_pool = ctx.enter_context(tc.tile_pool(name="vsb", bufs=2))
            x_pool = ctx.enter_context(tc.tile_pool(name="xbuf", bufs=1))
            tree_pool = ctx.enter_context(tc.tile_pool(name="tree", bufs=2))
            small_pool = ctx.enter_context(tc.tile_pool(name="small", bufs=2))
            vp_pool = ctx.enter_context(
                tc.tile_pool(name="vpsum", bufs=2, space="PSUM")
            )
            conv_pool = ctx.enter_context(
                tc.tile_pool(name="convpsum", bufs=2, space="PSUM")
            )
            w2p_pool = ctx.enter_context(
                tc.tile_pool(name="w2psum", bufs=1, space="PSUM")
            )
            misc_psum = ctx.enter_context(
                tc.tile_pool(name="miscpsum", bufs=1, space="PSUM")
            )

            # ---- load constants ----
            r124 = consts.tile([128, C1 * 124], BF16)
            nc.sync.dma_start(r124[:], r124_d)
            r52 = consts.tile([56, C1 * 52], BF16)
            nc.sync.dma_start(r52[:], r52_d)
            xi = consts.tile([128, NTILES], I32)
            nc.sync.dma_start(xi[:], xi_d)
            xn = consts.tile([128, NTILES], I32)
            nc.sync.dma_start(xn[:], xn_d)
            w2t = consts.tile([C1, K * C2], BF16)
            nc.sync.dma_start(w2t[:], w2t_d)
            wl = consts.tile([C2, 5], BF16)
            nc.sync.dma_start(wl[:], wl_d)
            b1bc = consts.tile([128, C1], F32)
            nc.sync.dma_start(b1bc[:], b1bc_d)
            b2c = consts.tile([128, 1], F32)
            nc.sync.dma_start(b2c[:], b2c_d)
            blc = consts.tile([1, 5], F32)
            nc.sync.dma_start(blc[:], blc_d)

            ident = consts.tile([128, 128], BF16)
            make_identity(nc, ident[:])

            # persistent ping-pong max-tree buffers; only the pad strips
            # ([124:128] and [252:256] per channel) need NEG — the data
            # regions are fully rewritten every tile. Memset on DVE so the
            # gpsimd SWDGE queue is free to emit gather descriptors.
            xbuf0 = x_pool.tile([128, C1 * 256], BF16)
            xbuf1 = x_pool.tile([128, C1 * 256], BF16)
            for xb_ in (xbuf0, xbuf1):
                xv = xb_[:].rearrange("p (c d) -> p c d", c=C1)
                nc.vector.memset(xv[:, :, 124:128], NEG)
                nc.vector.memset(xv[:, :, 252:256], NEG)

            out1_all = consts.tile([128, NTILES * C1], BF16)
            out1_3 = out1_all[:].rearrange("p (t c) -> p t c", t=NTILES)

            # ---- conv1 + maxpool over 12 token tiles ----
            for i in range(NTILES):
                E = e_pool.tile([128, 304], BF16, tag="E")
                nc.gpsimd.indirect_dma_start(
                    out=E[:, 0:300],
                    out_offset=None,
                    in_=emb16,
                    in_offset=bass.IndirectOffsetOnAxis(ap=xi[:, i : i + 1], axis=0),
                )
                nc.gpsimd.indirect_dma_start(
                    out=E[:, 300:304],
                    out_offset=None,
                    in_=emb4,
                    in_offset=bass.IndirectOffsetOnAxis(ap=xn[:, i : i + 1], axis=0),
                )

                # transpose the three d-windows: V[d, tok]
                Vp = vp_pool.tile([128, 384], BF16, tag="vp", space="PSUM")
                nc.tensor.transpose(Vp[:, 0:128], E[:, 0:128], ident[:])
                nc.tensor.transpose(Vp[:, 128:256], E[:, 124:252], ident[:])
                nc.tensor.transpose(Vp[0:56, 256:384], E[:, 248:304], ident[:])
                V = vsb_pool.tile([128, 384], BF16, tag="V")
                nc.scalar.copy(V[:, 0:256], Vp[:, 0:256])
                nc.scalar.copy(V[0:56, 256:384], Vp[0:56, 256:384])

                xb = xbuf0 if i % 2 == 0 else xbuf1
                x3 = xb[:].rearrange("p (c d) -> p c d", c=C1)  # [128, 32, 256]

                w2dir = small_pool.tile([128, C1], F32, tag="w2dir")

                for u in range(8):  # 4-channel units: [w0-bank | w1-bank]
                    c0 = 4 * u
                    P01 = conv_pool.tile([128, 1024], F32, tag="cp", space="PSUM")
                    nc.tensor.matmul(
                        P01[:, 0:496], V[:, 0:128],
                        r124[:, c0 * 124 : (c0 + 4) * 124], start=True, stop=True,
                    )
                    nc.tensor.matmul(
                        P01[:, 512:1008], V[:, 128:256],
                        r124[:, c0 * 124 : (c0 + 4) * 124], start=True, stop=True,
                    )
                    # one ACT copy drains both windows for 4 channels
                    # psum: [w=2 banks][c=4 x d=124 compact][pad]
                    pv = (
                        P01[:]
                        .rearrange("p (w z) -> p w z", w=2)[:, :, 0:496]
                        .rearrange("p w (c d) -> p w c d", c=4)
                    )
                    dst = x3[:, c0 : c0 + 4, :].rearrange(
                        "p c (w z) -> p w c z", w=2
                    )[:, :, :, 0:124]
                    nc.scalar.copy(dst, pv)

                for u in range(4):
                    c0 = 8 * u
                    # window 2 (d 248..299): 8 channels x 52
                    P2 = w2p_pool.tile([128, 512], F32, tag="w2p", space="PSUM")
                    nc.tensor.matmul(
                        P2[:, 0:416], V[0:56, 256:384],
                        r52[:, c0 * 52 : (c0 + 8) * 52], start=True, stop=True,
                    )
                    # DVE direct-reduces window 2 (8 channels x 52, compact)
                    p2v = P2[:, 0:416].rearrange("p (c d) -> p c d", c=8)
                    nc.vector.reduce_max(
                        w2dir[:, c0 : c0 + 8], p2v, axis=mybir.AxisListType.X,
                    )

                # max tree over d (bf16, SBUF, all 2x-mode)
                l1 = tree_pool.tile([128, C1 * 128], BF16, tag="l1")
                l1v = l1[:].rearrange("p (c d) -> p c d", c=C1)
                nc.vector.tensor_tensor(
                    l1v, x3[:, :, 0:128], x3[:, :, 128:256], op=mybir.AluOpType.max
                )
                l2 = tree_pool.tile([128, C1 * 64], BF16, tag="l2")
                l2v = l2[:].rearrange("p (c d) -> p c d", c=C1)
                nc.vector.tensor_tensor(
                    l2v, l1v[:, :, 0:64], l1v[:, :, 64:128], op=mybir.AluOpType.max
                )
                l3 = tree_pool.tile([128, C1 * 32], BF16, tag="l3")
                l3v = l3[:].rearrange("p (c d) -> p c d", c=C1)
                nc.vector.tensor_tensor(
                    l3v, l2v[:, :, 0:32], l2v[:, :, 32:64], op=mybir.AluOpType.max
                )
                l4 = tree_pool.tile([128, C1 * 16], BF16, tag="l4")
                l4v = l4[:].rearrange("p (c d) -> p c d", c=C1)
                nc.vector.tensor_tensor(
                    l4v, l3v[:, :, 0:16], l3v[:, :, 16:32], op=mybir.AluOpType.max
                )
                if debug and i == 0:
                    nc.sync.dma_start(dbg_e, E[:])
                    nc.sync.dma_start(dbg_v, V[:])
                    nc.sync.dma_start(dbg_x, xb[:])
                    nc.sync.dma_start(dbg_w2, w2dir[:])

                red = small_pool.tile([128, C1], BF16, tag="red")
                nc.vector.reduce_max(red[:], l4v, axis=mybir.AxisListType.X)

                cmb = small_pool.tile([128, C1], F32, tag="cmb")
                nc.vector.tensor_tensor(
                    cmb[:], red[:], w2dir[:], op=mybir.AluOpType.max
                )
                cmb2 = small_pool.tile([128, C1], F32, tag="cmb2")
                nc.vector.tensor_tensor(
                    cmb2[:], cmb[:], b1bc[:], op=mybir.AluOpType.add
                )
                nc.vector.tensor_scalar_max(out1_3[:, i, :], cmb2[:], 0.0)

            # ---- transpose out1 -> [32, 1536] ----
            out1t = consts.tile([C1, TOKPAD], BF16)
            for i in range(NTILES):
                Tp = vp_pool.tile([C1, 128], BF16, tag="vp", space="PSUM")
                nc.tensor.transpose(Tp[:], out1_3[:, i, :], ident[:])
                nc.vector.tensor_copy(out1t[:, i * 128 : (i + 1) * 128], Tp[:])

            # ---- conv2 + maxpool(1500) ----
            hpart = consts.tile([128, 4], F32)
            for j in range(3):
                Cp = misc_psum.tile([C2, 512], F32, tag="mp", space="PSUM")
                for k in range(K):
                    nc.tensor.matmul(
                        Cp[:, 0:500],
                        w2t[:, k * C2 : (k + 1) * C2],
                        out1t[:, j * 500 + k : j * 500 + k + 500],
                        start=(k == 0),
                        stop=(k == K - 1),
                    )
                nc.vector.reduce_max(
                    hpart[:, j : j + 1], Cp[:, 0:500], axis=mybir.AxisListType.X
                )

            if debug:
                nc.sync.dma_start(dbg_o1, out1_all[:])
                nc.sync.dma_start(dbg_o1t, out1t[:])
                nc.sync.dma_start(dbg_hp, hpart[:])

            hmax = consts.tile([128, 1], F32)
            nc.vector.reduce_max(hmax[:], hpart[:, 0:3], axis=mybir.AxisListType.X)
            hb = consts.tile([128, 1], F32)
            nc.vector.tensor_scalar_add(hb[:], hmax[:], b2c[:, 0:1])
            h16 = consts.tile([128, 1], BF16)
            nc.vector.tensor_scalar_max(h16[:], hb[:], 0.0)

            # ---- linear + softmax ----
            Fp = misc_psum.tile([1, 512], F32, tag="mp", space="PSUM")
            nc.tensor.matmul(Fp[0:1, 0:5], h16[:, 0:1], wl[:, 0:5], start=True, stop=True)
            lg = consts.tile([1, 8], F32)
            nc.vector.tensor_tensor(
                lg[:, 0:5], Fp[0:1, 0:5], blc[0:1, 0:5], op=mybir.AluOpType.add
            )
            mx = consts.tile([1, 1], F32)
            nc.vector.reduce_max(mx[:], lg[:, 0:5], axis=mybir.AxisListType.X)
            sh = consts.tile([1, 8], F32)
            nc.vector.tensor_scalar(
                sh[:, 0:5], lg[:, 0:5], mx[0:1, 0:1], None,
                op0=mybir.AluOpType.subtract,
            )
            ex = consts.tile([1, 8], F32)
            nc.scalar.activation(
                ex[:, 0:5], sh[:, 0:5], mybir.ActivationFunctionType.Exp
            )
            sm = consts.tile([1, 1], F32)
            nc.vector.reduce_sum(sm[:], ex[:, 0:5], axis=mybir.AxisListType.X)
            rc = consts.tile([1, 1], F32)
            nc.vector.reciprocal(rc[:], sm[:])
            res = consts.tile([1, 8], F32)
            nc.vector.tensor_scalar_mul(res[:, 0:5], ex[:, 0:5], rc[0:1, 0:1])
            nc.sync.dma_start(out_d, res[:, 0:5])
            if debug:
                nc.sync.dma_start(dbg_lg, lg[:])

    nc.compile()
    return nc


def _prep_shared(emb, w1, b1, w2, b2, wl, bl):
    bf = ml_dtypes.bfloat16
    emb = np.asarray(emb, np.float32)
    emb16 = emb.astype(bf)
    emb4 = np.ascontiguousarray(emb16[:, 0:4])

    w1 = np.asarray(w1, np.float32).reshape(C1, K)
    r124 = np.zeros((128, C1 * 124), np.float32)
    i_idx = np.arange(128)[:, None]
    for c in range(C1):
        for d in range(124):
            for k in range(K):
                r124[d + k, c * 124 + d] = w1[c, k]
    r52 = np.zeros((56, C1 * 52), np.float32)
    for c in range(C1):
        for d in range(52):
            for k in range(K):
                r52[d + k, c * 52 + d] = w1[c, k]

    w2 = np.asarray(w2, np.float32)  # [C2, C1, K]
    w2t = np.zeros((C1, K * C2), np.float32)
    for k in range(K):
        w2t[:, k * C2 : (k + 1) * C2] = w2[:, :, k].T
    wlT = np.asarray(wl, np.float32).T  # [C2, 5]
    b1bc = np.broadcast_to(np.asarray(b1, np.float32)[None, :], (128, C1)).copy()
    b2c = np.broadcast_to(
        np.asarray(b2, np.float32)[:, None], (C2, 1)
    ).copy()  # [128,1] per-partition
    blc = np.asarray(bl, np.float32).reshape(1, 5).copy()

    return {
        "emb16": np.ascontiguousarray(emb16),
        "emb4": emb4,
        "r124": r124.astype(bf),
        "r52": r52.astype(bf),
        "w2t": w2t.astype(bf),
        "wl": wlT.astype(bf),
        "b1bc": b1bc,
        "b2c": b2c,
        "blc": blc,
    }


def kernel(**inputs) -> np.ndarray:
    global LAST_RESULT
    x = np.asarray(inputs["x"]).astype(np.int64)
    shared = _prep_shared(
        inputs["emb"], inputs["w1"], inputs["b1"], inputs["w2"],
        inputs["b2"], inputs["wl"], inputs["bl"],
    )

    key = ("dbg" if DEBUG else "nc")
    if key not in _cached:
        _cached[key] = _build_program(debug=DEBUG)
    nc = _cached[key]

    in_maps = []
    for b in range(B):
        xi = x[b, 0:TOKPAD].reshape(NTILES, 128).T.astype(np.int32)
        xn = x[b, 1 : TOKPAD + 1].reshape(NTILES, 128).T.astype(np.int32)
        m = dict(shared)
        m["xi"] = np.ascontiguousarray(xi)
        m["xn"] = np.ascontiguousarray(xn)
        in_maps.append(m)

    r = run_bass_kernel_spmd(nc, in_maps, list(range(B)), trace=TRACE)
    LAST_RESULT = r
    out = np.stack([np.asarray(r.results[b]["out"], np.float32)[0] for b in range(B)])
    return out


if __name__ == "__main__":
    rng = np.random.default_rng(0)
    ins = {
        "x": rng.integers(0, VOCAB, (B, SEQ)),
        "emb": rng.standard_normal((VOCAB, D), np.float32) * 0.1,
        "w1": rng.standard_normal((C1, 1, K), np.float32) * 0.1,
        "b1": np.zeros(C1, np.float32),
        "w2": rng.standard_normal((C2, C1, K), np.float32) * 0.05,
        "b2": np.zeros(C2, np.float32),
        "wl": rng.standard_normal((5, C2), np.float32) * 0.1,
        "bl": np.zeros(5, np.float32),
    }
    print(kernel(**ins))
